# revision 1
# baseline (speedup 1.0000x reference)
"""DegreeGCNPlusLayer for Trainium2 (Bass/Tile), 8-core SPMD.

Computes: out = (segment_sum(inputs[src], dst) / degree[:, None]) @ W + b

Strategy (hardcoded for N=100000, E=640000, D=128, 8 cores):
  - Nodes sharded 12500/core (row blocks); `inputs` replicated to every
    core's HBM; edges partitioned by dst ownership.
  - Per core, edges sorted by (dst tile, src block, src). For each 128-node
    dst tile, source rows are fetched with dma_gather (int16 indices => 4
    src blocks of 25000 rows), 128 rows per matmul chunk; pad slots are
    skipped at runtime via num_idxs_reg and trailing -1 indices.
  - Scatter-add realized as PE matmuls: psum[dst,feat] += onehot^T @ msgs,
    with the one-hot built on the DVE from local-dst metadata via iota
    compare (pad slots get sentinel 999 -> zero one-hot row, which also
    nullifies garbage rows in the gather buffer).
  - Epilogue per tile (all on-chip): invdeg row-scale (DVE), PE transpose,
    out2T = W^T @ hT (PE), bias add (ACT), DMA out. Output is stored
    transposed per core ([128 feat, 12544 nodes]); the host reassembles.
"""

import math

import numpy as np

N_NODES = 100000
N_EDGES = 640000
D = 128
N_CORES = 8
NPC = N_NODES // N_CORES          # 12500 nodes per core
P = 128
NT = math.ceil(NPC / P)           # 98 dst tiles per core
NBLK = 4
BLK = 25000                       # src rows per int16-addressable block
PAD_NT = NT * P                   # 12544 padded nodes per core

_CACHE = {}


def _prepare(src, dst, degree):
    """Host-side sharding metadata -> (KB, per-core dict of arrays)."""
    order0 = np.argsort(dst, kind="stable")
    src_s = src[order0]
    dst_s = dst[order0]
    core_of = dst_s // NPC
    core_bounds = np.searchsorted(core_of, np.arange(N_CORES + 1))

    per_core = []
    kb_max = 1
    for c in range(N_CORES):
        lo, hi = core_bounds[c], core_bounds[c + 1]
        s = src_s[lo:hi].astype(np.int64)
        d = dst_s[lo:hi].astype(np.int64) - c * NPC
        tile_id = d // P
        blk_id = s // BLK
        o = np.lexsort((s, blk_id, tile_id))
        s, d, tile_id, blk_id = s[o], d[o], tile_id[o], blk_id[o]
        cell = tile_id * NBLK + blk_id
        counts = np.bincount(cell, minlength=NT * NBLK)
        per_core.append((s, d, cell, counts))
        kb_max = max(kb_max, counts.max())
    KB = max(1, math.ceil(kb_max / P))
    S = KB * P
    CH = NBLK * KB
    CALLS = NT * NBLK

    cores = []
    for c in range(N_CORES):
        s, d, cell, counts = per_core[c]
        starts = np.zeros(NT * NBLK + 1, np.int64)
        np.cumsum(counts, out=starts[1:])
        pos = np.arange(len(s)) - starts[cell]

        idx_slots = np.full((CALLS, S), -1, np.int16)
        idx_slots[cell, pos] = (s - (cell % NBLK) * BLK).astype(np.int16)
        cnts = np.maximum(counts, 1).astype(np.int32)
        idx_slots[counts == 0, 0] = 0   # dummy valid idx, sentinel ldst

        # ldst_pc[p, t*CH + g] = local dst of tile-slot g*128+p (sentinel 999)
        ldst_pc = np.full((P, NT * CH), 999.0, np.float32)
        slot = (cell % NBLK) * S + pos            # slot within the tile
        t_of = cell // NBLK
        ldst_pc[slot % P, t_of * CH + slot // P] = (d - t_of * P).astype(np.float32)

        # dma_gather index layout: idx i of call j -> [i % 16, i // 16],
        # replicated across the 8 Q7 groups (partition rows 0..127)
        wrapped = idx_slots.reshape(CALLS, S // 16, 16).transpose(2, 0, 1)
        idxw = np.tile(wrapped.reshape(16, CALLS * (S // 16)), (8, 1)).copy()

        iv = np.ones(PAD_NT, np.float32)
        iv[:NPC] = 1.0 / degree[c * NPC : (c + 1) * NPC]
        invdeg = np.ascontiguousarray(iv.reshape(NT, P).T)

        cores.append({
            "idxw": idxw,
            "ldst": ldst_pc,
            "counts": cnts.reshape(1, CALLS),
            "invdeg": invdeg,
        })
    return KB, cores


def _build(KB, with_reps=False):
    import concourse.tile as tile
    from concourse import bacc, mybir

    S = KB * P
    CALLS = NT * NBLK
    CH = NBLK * KB                  # matmul chunks per tile

    nc = bacc.Bacc("TRN2", target_bir_lowering=False, debug=False,
                   enable_asserts=False, num_devices=N_CORES,
                   num_swdge_queues=4)
    f32, i32, i16 = mybir.dt.float32, mybir.dt.int32, mybir.dt.int16
    t_inputs = nc.dram_tensor("inputs", [N_NODES, D], f32, kind="ExternalInput").ap()
    t_w = nc.dram_tensor("W", [D, D], f32, kind="ExternalInput").ap()
    t_ident = nc.dram_tensor("ident", [P, P], f32, kind="ExternalInput").ap()
    t_b = nc.dram_tensor("b", [P, 1], f32, kind="ExternalInput").ap()
    t_iota = nc.dram_tensor("iota", [P, CH * P], f32, kind="ExternalInput").ap()
    t_idxw = nc.dram_tensor("idxw", [P, CALLS * (S // 16)], i16, kind="ExternalInput").ap()
    t_ldst = nc.dram_tensor("ldst", [P, NT * CH], f32, kind="ExternalInput").ap()
    t_counts = nc.dram_tensor("counts", [1, CALLS], i32, kind="ExternalInput").ap()
    t_invdeg = nc.dram_tensor("invdeg", [P, NT], f32, kind="ExternalInput").ap()
    t_out = nc.dram_tensor("outT", [P, PAD_NT], f32, kind="ExternalOutput").ap()
    if with_reps:
        t_reps = nc.dram_tensor("reps", [1, 1], i32, kind="ExternalInput").ap()

    blocks = [t_inputs[blk * BLK : min((blk + 1) * BLK, N_NODES), :]
              for blk in range(NBLK)]

    with tile.TileContext(nc) as tc:
        with (
            tc.tile_pool(name="meta", bufs=1) as meta,
            tc.tile_pool(name="gbuf", bufs=5) as gpool,
            tc.tile_pool(name="oh", bufs=4) as ohpool,
            tc.tile_pool(name="ep", bufs=3) as eppool,
            tc.tile_pool(name="ph", bufs=4, space="PSUM") as ph,
            tc.tile_pool(name="pt", bufs=2, space="PSUM") as pt,
            tc.tile_pool(name="po", bufs=2, space="PSUM") as po,
        ):
            idx_sb = meta.tile([P, CALLS * (S // 16)], i16)
            nc.sync.dma_start(idx_sb[:], t_idxw[:])
            ldst_sb = meta.tile([P, NT * CH], f32)
            nc.sync.dma_start(ldst_sb[:], t_ldst[:])
            counts_sb = meta.tile([1, CALLS], i32)
            nc.sync.dma_start(counts_sb[:], t_counts[:])
            iota_sb = meta.tile([P, CH * P], f32)
            nc.sync.dma_start(iota_sb[:], t_iota[:])
            invdeg_sb = meta.tile([P, NT], f32)
            nc.sync.dma_start(invdeg_sb[:], t_invdeg[:])
            w_sb = meta.tile([D, D], f32)
            nc.sync.dma_start(w_sb[:], t_w[:])
            ident_sb = meta.tile([P, P], f32)
            nc.sync.dma_start(ident_sb[:], t_ident[:])
            b_sb = meta.tile([P, 1], f32)
            nc.sync.dma_start(b_sb[:], t_b[:])

            Pool = mybir.EngineType.Pool
            cregs = [nc.alloc_register(Pool, f"cnt{i}") for i in range(8)]

            def body():
                for t in range(NT):
                    gbuf = gpool.tile([P, CH, P], f32, tag="g")
                    for blk in range(NBLK):
                        j = t * NBLK + blk
                        reg = cregs[j % len(cregs)]
                        nc.reg_load(reg, counts_sb[0:1, j : j + 1])
                        nc.gpsimd.dma_gather(
                            out_ap=gbuf[:, blk * KB : (blk + 1) * KB, :],
                            in_ap=blocks[blk],
                            idxs_ap=idx_sb[:, j * (S // 16) : (j + 1) * (S // 16)],
                            num_idxs=S,
                            num_idxs_reg=reg,
                            elem_size=D,
                            queue_num=blk,
                            single_packet=False,
                        )
                    onehot = ohpool.tile([P, CH, P], f32, tag="oh")
                    nc.vector.tensor_tensor(
                        out=onehot[:],
                        in0=ldst_sb[:, t * CH : (t + 1) * CH, None].broadcast_to(
                            [P, CH, P]),
                        in1=iota_sb[:, :].rearrange("p (g j) -> p g j", j=P),
                        op=mybir.AluOpType.is_equal,
                    )
                    psum_h = ph.tile([P, P], f32, tag="h", space="PSUM")
                    for g in range(CH):
                        nc.tensor.matmul(
                            out=psum_h[:],
                            lhsT=onehot[:, g, :],
                            rhs=gbuf[:, g, :],
                            start=(g == 0),
                            stop=(g == CH - 1),
                        )
                    h_norm = eppool.tile([P, P], f32, tag="hn")
                    nc.vector.tensor_scalar_mul(h_norm[:], psum_h[:],
                                                invdeg_sb[:, t : t + 1])
                    psum_ht = pt.tile([P, P], f32, tag="ht", space="PSUM")
                    nc.tensor.transpose(out=psum_ht[:], in_=h_norm[:],
                                        identity=ident_sb[:])
                    ht_sb = eppool.tile([P, P], f32, tag="hts")
                    nc.scalar.copy(ht_sb[:], psum_ht[:])
                    psum_o = po.tile([P, P], f32, tag="o", space="PSUM")
                    nc.tensor.matmul(out=psum_o[:], lhsT=w_sb[:], rhs=ht_sb[:],
                                     start=True, stop=True)
                    out_sb = eppool.tile([P, P], f32, tag="os")
                    nc.scalar.activation(
                        out_sb[:], psum_o[:],
                        mybir.ActivationFunctionType.Identity,
                        bias=b_sb[:, 0:1],
                    )
                    nc.sync.dma_start(t_out[:, t * P : (t + 1) * P], out_sb[:])

            if with_reps:
                tmp = nc.alloc_registers("reps_regs")
                nc.regs_load(tmp, t_reps[0:1, 0:1])
                reps_val = nc.snap(tmp, donate=True, min_val=0, max_val=1 << 20)
                with tc.For_i(0, reps_val, 1):
                    body()
            else:
                body()

    nc.compile()
    return nc


def _iota_const(KB):
    return np.tile(np.arange(P, dtype=np.float32), (P, NBLK * KB))


def make_in_maps(inputs, W, b, KB, cores):
    iota = _iota_const(KB)
    ident = np.eye(P, dtype=np.float32)
    b_col = np.ascontiguousarray(b.reshape(P, 1))
    in_maps = []
    for c in range(N_CORES):
        m = cores[c]
        in_maps.append({
            "inputs": inputs,
            "W": W,
            "ident": ident,
            "b": b_col,
            "iota": iota,
            "idxw": m["idxw"],
            "ldst": m["ldst"],
            "counts": m["counts"],
            "invdeg": m["invdeg"],
        })
    return in_maps


def kernel(inputs, src, dst, degree, W, b):
    from concourse import bass_utils

    inputs = np.ascontiguousarray(np.asarray(inputs, dtype=np.float32))
    src = np.asarray(src).astype(np.int64)
    dst = np.asarray(dst).astype(np.int64)
    degree = np.asarray(degree, dtype=np.float32)
    W = np.ascontiguousarray(np.asarray(W, dtype=np.float32))
    b = np.asarray(b, dtype=np.float32)

    KB, cores = _prepare(src, dst, degree)
    if KB not in _CACHE:
        _CACHE[KB] = _build(KB, with_reps=False)
    nc = _CACHE[KB]

    in_maps = make_in_maps(inputs, W, b, KB, cores)
    res = bass_utils.run_bass_kernel_spmd(nc, in_maps, core_ids=list(range(N_CORES)))
    out = np.empty((N_NODES, D), np.float32)
    for c in range(N_CORES):
        out[c * NPC : (c + 1) * NPC] = res.results[c]["outT"].T[:NPC]
    return out

